# revision 2
# baseline (speedup 1.0000x reference)
"""Trainium2 Bass kernel for nn_AttentionBiasHead — v2 (transposed dataflow).

Strategy (8 NeuronCores, data-parallel over batch, Wb2 column-sharded):

- Bias pipeline first: Wb2 shard streamed on the sync DMA queue at full
  bandwidth; GEMM (M=32, PSUM packed 4 chunks/bank via tile_position) chases
  the stream so the AllToAll trigger fires ~30us, before the CC rendezvous
  barrier ends (~67us).
- Attention uses a transposed dataflow: S^T = k^T-chunks x q^T gives
  E^T = exp(S^T) with m on partitions, which feeds O1^T = v^T... =
  matmul(lhsT=v[mc,:], rhs=E^T[mc,:]) directly — no attn transposes.
  Row sums via ones-matmul (M=1); normalization folded into O1 pre-A2A
  (O1n = O1 * broadcast(1/rowsum)).
- Tail after the A2A is only: bias DMA -> 16 PE transposes -> +bb2T copy ->
  4 matmuls (B^T @ v into fresh PSUM) -> one tensor_tensor add -> out DMA.
"""

import numpy as np

N_CORES = 8
B, L, DIN, DQ, DS, DMLP = 32, 512, 512, 128, 256, 128
BPC = B // N_CORES          # samples per core = 4
NSH = L * L // N_CORES      # bias-shard columns per core = 32768
NG = 4                      # Wb2 chunks per DMA group
NGRP = NSH // 512 // NG     # DMA groups = 8
KT = DIN // 128             # contraction tiles for projections = 4
NC = L // 128               # 128-chunks per l dim = 4
SCALE = 1.0 / float(np.sqrt(DQ))

_cache = {}


def _build():
    if "nc" in _cache:
        return _cache["nc"]

    from contextlib import ExitStack

    import concourse.mybir as mybir
    import concourse.tile as tile
    from concourse import bacc
    from concourse.bass import ts, _add_dep_helper
    from concourse.masks import make_identity

    dt = mybir.dt
    f32, f16, u8 = dt.float32, dt.float16, dt.uint8

    nc = bacc.Bacc("TRN2", target_bir_lowering=False, debug=False,
                   num_devices=N_CORES)

    # ---- per-core external tensors -------------------------------------
    qT_d = nc.dram_tensor("qT", [BPC, 128, KT, L], f16, kind="ExternalInput").ap()
    kT_d = nc.dram_tensor("kT", [BPC, 128, KT, L], f16, kind="ExternalInput").ap()
    vT_d = nc.dram_tensor("vT", [BPC, 128, KT, L], f16, kind="ExternalInput").ap()
    mkT_d = nc.dram_tensor("mkT", [BPC, 128, NC, L], u8, kind="ExternalInput").ap()
    sfT_d = nc.dram_tensor("sfT", [128, DS // 128, B], f32, kind="ExternalInput").ap()
    wqkv_d = nc.dram_tensor("wqkv", [128, KT, 3, DQ], f16, kind="ExternalInput").ap()
    bias4_d = nc.dram_tensor("bias4", [128, 4], f32, kind="ExternalInput").ap()
    Wb1_d = nc.dram_tensor("Wb1", [128, DS // 128, DMLP], f32, kind="ExternalInput").ap()
    Wb2s_d = nc.dram_tensor("Wb2s", [DMLP, NSH], f16, kind="ExternalInput").ap()
    bb2T_d = nc.dram_tensor("bb2T", [128, NC, L], f16, kind="ExternalInput").ap()
    out_d = nc.dram_tensor("out", [BPC, L, DQ], f16, kind="ExternalOutput").ap()

    with tile.TileContext(nc) as tc, ExitStack() as ctx:
        consts = ctx.enter_context(tc.tile_pool(name="consts", bufs=1))
        dram = ctx.enter_context(tc.tile_pool(name="dram", bufs=1, space="DRAM"))

        # ---- const loads (gpsimd queue; bias pipeline tensors first) ----
        sfT_sb = consts.tile([128, DS // 128, B], f32)
        nc.gpsimd.dma_start(sfT_sb[:], sfT_d[:])
        Wb1_sb = consts.tile([128, DS // 128, DMLP], f32)
        nc.gpsimd.dma_start(Wb1_sb[:], Wb1_d[:])
        bias4_sb = consts.tile([128, 4], f32)
        nc.gpsimd.dma_start(bias4_sb[:], bias4_d[:])
        wqkv_sb = consts.tile([128, KT, 3, DQ], f16)
        nc.gpsimd.dma_start(wqkv_sb[:], wqkv_d[:])
        bb2T_sb = consts.tile([128, NC, L], f16)
        nc.gpsimd.dma_start(bb2T_sb[:], bb2T_d[:])

        ident16 = consts.tile([128, 128], f16)
        make_identity(nc, ident16)
        c1 = consts.tile([128, 1], f16)
        nc.vector.memset(c1, 1.0)
        ones16 = consts.tile([128, 1], f16)
        nc.vector.memset(ones16, 1.0)

        # ---- shared PSUM pools -----------------------------------------
        # pool B: fast-rotating scratch (proj, S^T, rowsum, transposes)
        psB = ctx.enter_context(tc.tile_pool(name="psB", bufs=4, space="PSUM"))

        # ---- phase A: H^T = relu(Wb1^T @ sf^T + bb1)  [128, 32] --------
        ht_ps = psB.tile([128, 512], f32, tag="ps", name="ht_ps")
        for kt in range(DS // 128):
            nc.tensor.matmul(ht_ps[:, :B], Wb1_sb[:, kt], sfT_sb[:, kt],
                             start=(kt == 0), stop=(kt == DS // 128 - 1))
        HT_sb = consts.tile([128, B], f16)
        nc.scalar.activation(HT_sb[:], ht_ps[:, :B],
                             mybir.ActivationFunctionType.Relu,
                             bias=bias4_sb[:, 3:4], scale=1.0)

        # ---- phase B: bias shard GEMM + AllToAll -----------------------
        a2a_in = dram.tile([B, NSH], f16)
        a2a_out = dram.tile([B, NSH], f16)

        bsbp = ctx.enter_context(tc.tile_pool(name="bsb", bufs=2))
        w2p_cm = tc.tile_pool(name="w2", bufs=16)
        w2p = w2p_cm.__enter__()
        w2ts = []
        w2ds = []
        for g in range(NGRP):
            w2t = w2p.tile([128, NG, 512], f16, tag="w2t", name=f"w2t{g}")
            w2d = nc.sync.dma_start(
                w2t[:], Wb2s_d[:, ts(g, NG * 512)].rearrange(
                    "p (n w) -> p n w", w=512))
            w2ts.append(w2t)
            w2ds.append(w2d)

        # a2a_in row (for dest i, sample 4i+sl) = 4i+sl = global sample id.
        # GEMM pair p covers chunks q=2p, 2p+1 -> psum [32, 2, 512].
        a2a_in_v = a2a_in.rearrange("s (t g w) -> t s g w", g=8, w=512)
        copy_engs = [nc.vector.tensor_copy, nc.scalar.copy]
        with tc.tile_pool(name="bpsp", bufs=2, space="PSUM") as bpsp:
            for t in range(8):
                bsb = bsbp.tile([32, 8, 512], f16, tag="bsb", name=f"bsb{t}")
                for u in range(4):
                    bps = bpsp.tile([32, 2, 512], f32, tag="bps",
                                    name=f"bps{t}_{u}")
                    for g2 in range(2):
                        q = 8 * t + 2 * u + g2
                        mm = nc.tensor.matmul(bps[:, g2], HT_sb[:],
                                              w2ts[q // NG][:, q % NG],
                                              start=True, stop=True)
                        if t == 0 and u == 0 and g2 == 0:
                            _add_dep_helper(
                                mm.ins, w2ds[7].ins, sync=True,
                                reason="blast GEMM only once half of Wb2 "
                                       "is resident (PE p-state ramp)")
                    copy_engs[u % 2](bsb[:, ts(u, 2)], bps[:])
                wr = nc.sync.dma_start(a2a_in_v[t], bsb[:])
            last_write = wr
        w2p_cm.__exit__(None, None, None)
        # pool A: long-lived per-sample accumulators (O1 pre-A2A, O2 tail);
        # created after the GEMM pool closes so PSUM fits.
        psA = ctx.enter_context(tc.tile_pool(name="psA", bufs=4, space="PSUM"))

        nc.gpsimd.collective_compute(
            "AllToAll", mybir.AluOpType.bypass,
            replica_groups=[list(range(N_CORES))],
            ins=[a2a_in.opt()], outs=[a2a_out.opt()],
        )
        # a2a_out row (j, sl): from source j = cols l1 in [64j, 64j+64).
        a2a_v = a2a_out.rearrange("(c2 hi sl) (l1l l2) -> c2 sl hi l1l l2",
                                  c2=NC, hi=2, l2=L)

        # ---- input loads: qk on sync (behind Wb2 stream), v/mask gpsimd
        inp = ctx.enter_context(tc.tile_pool(name="inp", bufs=2 * BPC))
        vinp = ctx.enter_context(tc.tile_pool(name="vinp", bufs=BPC))
        mskp = ctx.enter_context(tc.tile_pool(name="mskp", bufs=BPC))
        qTin, kTin, vTin, mtile = {}, {}, {}, {}
        for s in range(BPC):
            qTin[s] = inp.tile([128, KT, L], f16, tag="qTin", name=f"qTin{s}")
            qd = nc.scalar.dma_start(qTin[s][:], qT_d[s])
            _add_dep_helper(qd.ins, w2ds[-1].ins, sync=True,
                            reason="defer q loads behind Wb2 stream")
            kTin[s] = inp.tile([128, KT, L], f16, tag="kTin", name=f"kTin{s}")
            kd = nc.scalar.dma_start(kTin[s][:], kT_d[s])
            _add_dep_helper(kd.ins, w2ds[-1].ins, sync=True,
                            reason="defer k loads behind Wb2 stream")
        for s in range(BPC):
            vTin[s] = vinp.tile([128, KT, L], f16, tag="vTin", name=f"vTin{s}")
            vd = nc.scalar.dma_start(vTin[s][:], vT_d[s])
            _add_dep_helper(vd.ins, w2ds[-1].ins, sync=True,
                            reason="defer v loads behind Wb2 stream")
            mtile[s] = mskp.tile([128, NC, L], u8, tag="mt", name=f"mt{s}")
            md = nc.scalar.dma_start(mtile[s][:], mkT_d[s])
            _add_dep_helper(md.ins, w2ds[-1].ins, sync=True,
                            reason="defer mask loads behind Wb2 stream")

        # ---- phase C: projections --------------------------------------
        prj = ctx.enter_context(tc.tile_pool(name="prj", bufs=2 * BPC))
        vpool = ctx.enter_context(tc.tile_pool(name="vpool", bufs=BPC))
        qT_t, kT_t, v_t = {}, {}, {}
        for s in range(BPC):
            q_ps = psB.tile([128, 512], f32, tag="ps", name=f"qps{s}")
            for kt in range(KT):
                nc.tensor.matmul(q_ps[:], wqkv_sb[:, kt, 0], qTin[s][:, kt],
                                 start=(kt == 0), stop=(kt == KT - 1))
            qT_sb = prj.tile([128, L], f16, tag="qT", name=f"qT{s}")
            nc.vector.tensor_scalar_add(qT_sb[:], q_ps[:], bias4_sb[:, 0:1])
            qT_t[s] = qT_sb

            k_ps = psB.tile([128, 512], f32, tag="ps", name=f"kps{s}")
            for kt in range(KT):
                nc.tensor.matmul(k_ps[:], wqkv_sb[:, kt, 1], kTin[s][:, kt],
                                 start=(kt == 0), stop=(kt == KT - 1))
            kT_sb = prj.tile([128, L], f16, tag="kT", name=f"kT{s}")
            nc.vector.tensor_scalar_add(kT_sb[:], k_ps[:], bias4_sb[:, 1:2])
            kT_t[s] = kT_sb

            w_ps = psB.tile([128, 512], f32, tag="ps", name=f"wps{s}")
            for kt in range(KT):
                nc.tensor.matmul(w_ps[:], wqkv_sb[:, kt, 2], vTin[s][:, kt],
                                 start=(kt == 0), stop=(kt == KT - 1))
            vT_sb = prj.tile([128, L], f16, tag="vTs", name=f"vTs{s}")
            nc.vector.tensor_scalar_add(vT_sb[:], w_ps[:], bias4_sb[:, 2:3])
            v_ps = psB.tile([128, 512], f16, tag="ps", name=f"vps{s}",
                            padded_shape=[128, 1024])
            for j in range(NC):
                nc.tensor.transpose(v_ps[:, ts(j, 128)], vT_sb[:, ts(j, 128)],
                                    ident16)
            v_sb = vpool.tile([128, NC, DQ], f16, tag="v", name=f"v{s}")
            nc.scalar.copy(v_sb[:], v_ps[:].rearrange("p (j d) -> p j d", j=NC))
            v_t[s] = v_sb

        # ---- phase D: S^T, exp, rowsum, O1 (all pre-A2A) ---------------
        expp = ctx.enter_context(tc.tile_pool(name="expp", bufs=BPC))
        rcpp = ctx.enter_context(tc.tile_pool(name="rcpp", bufs=1))
        rbp = ctx.enter_context(tc.tile_pool(name="rbp", bufs=2))
        o1p = ctx.enter_context(tc.tile_pool(name="o1p", bufs=BPC))
        rcp_all = rcpp.tile([1, BPC, 512], f32, name="rcp_all")
        rcp_inv = rcpp.tile([1, BPC, 512], f32, name="rcp_inv")
        O1n_t = {}
        for s in range(BPC):
            ET = expp.tile([128, NC, L], f16, tag="ET", name=f"ET{s}")
            for mc in range(NC):
                s_ps = psB.tile([128, 512], f32, tag="ps", name=f"sps{s}_{mc}")
                nc.tensor.matmul(s_ps[:], kT_t[s][:, ts(mc, 128)], qT_t[s][:],
                                 start=True, stop=True)
                nc.scalar.activation(ET[:, mc], s_ps[:],
                                     mybir.ActivationFunctionType.Exp,
                                     bias=0.0, scale=SCALE)
                nc.vector.copy_predicated(ET[:, mc], mtile[s][:, mc],
                                          c1[:].to_broadcast([128, 512]))
            rs_ps = psB.tile([128, 512], f32, tag="ps", name=f"rs{s}")
            o1_ps = psA.tile([128, 512], f32, tag="psA", name=f"o1{s}")
            for mc in range(NC):
                nc.tensor.matmul(rs_ps[0:1, :], ones16[:], ET[:, mc],
                                 start=(mc == 0), stop=(mc == NC - 1))
            for mc in range(NC):
                nc.tensor.matmul(o1_ps[:], v_t[s][:, mc], ET[:, mc],
                                 start=(mc == 0), stop=(mc == NC - 1))
            nc.vector.tensor_copy(rcp_all[0:1, s], rs_ps[0:1, :])
            nc.vector.reciprocal_approx_fast(rcp_inv[0:1, s], rcp_all[0:1, s])
            rsB = rbp.tile([128, 512], f32, tag="rcpB", name=f"rsB{s}")
            nc.gpsimd.partition_broadcast(rsB[:], rcp_inv[0:1, s])
            O1n = o1p.tile([128, L], f16, tag="O1n", name=f"O1n{s}")
            nc.vector.tensor_tensor(O1n[:], o1_ps[:], rsB[:],
                                    mybir.AluOpType.mult)
            O1n_t[s] = O1n

        # ---- phase E: post-A2A tail ------------------------------------
        bi = ctx.enter_context(tc.tile_pool(name="bi", bufs=BPC))
        btp = ctx.enter_context(tc.tile_pool(name="btp", bufs=4))
        outp = ctx.enter_context(tc.tile_pool(name="outp", bufs=2))

        bias16_t = {}
        for s in range(BPC):
            bias16_t[s] = bi.tile([128, NC, L], f16, tag="bias16",
                                  name=f"b16_{s}")
            eng = nc.sync if s % 2 == 0 else nc.scalar
            eng.dma_start(bias16_t[s][0:64], a2a_v[:, s, 0].transpose([1, 0, 2]))
            eng.dma_start(bias16_t[s][64:128], a2a_v[:, s, 1].transpose([1, 0, 2]))
        for s in range(BPC):
            o2_ps = psA.tile([128, 512], f32, tag="psA", name=f"o2{s}")
            bt_pss = []
            for l2c in range(NC):
                bt_ps = psB.tile([128, 512], f16, tag="ps", name=f"bt{s}_{l2c}",
                                 padded_shape=[128, 1024])
                for c2 in range(NC):
                    nc.tensor.transpose(bt_ps[:, ts(c2, 128)],
                                        bias16_t[s][:, c2, ts(l2c, 128)],
                                        ident16)
                bt_pss.append(bt_ps)
            bt_sbs = []
            for l2c in range(NC):
                bt_sb = btp.tile([128, L], f16, tag="bt", name=f"btsb{s}_{l2c}")
                nc.vector.tensor_tensor(bt_sb[:], bt_pss[l2c][:],
                                        bb2T_sb[:, l2c], mybir.AluOpType.add)
                bt_sbs.append(bt_sb)
            for l2c in range(NC):
                nc.tensor.matmul(o2_ps[:], v_t[s][:, l2c], bt_sbs[l2c][:],
                                 start=(l2c == 0), stop=(l2c == NC - 1))
            oT_sb = outp.tile([128, L], f16, tag="oT", name=f"oTs{s}")
            nc.vector.tensor_tensor(oT_sb[:], o2_ps[:], O1n_t[s][:],
                                    mybir.AluOpType.add)
            o_sb = outp.tile([128, NC, DQ], f16, tag="o", name=f"os{s}")
            nc.sync.dma_start_transpose(o_sb[:], oT_sb[:])
            nc.sync.dma_start(out_d[s].rearrange("(j p) d -> p j d", p=128),
                              o_sb[:])

    nc.compile()
    _cache["nc"] = nc
    return nc


def _prep_in_maps(query, key, value, sf, atten_mask, Wq, bq, Wk, bk, Wv, bv,
                  Wb1, bb1, Wb2, bb2):
    f16 = np.float16
    sfT = np.ascontiguousarray(
        np.asarray(sf, np.float32).T.reshape(2, 128, B).transpose(1, 0, 2))
    wqkv = np.ascontiguousarray(
        np.stack([np.asarray(Wq, f16), np.asarray(Wk, f16),
                  np.asarray(Wv, f16)], axis=1)
        .reshape(KT, 128, 3, DQ).transpose(1, 0, 2, 3))
    bias4 = np.ascontiguousarray(
        np.stack([np.asarray(bq, np.float32), np.asarray(bk, np.float32),
                  np.asarray(bv, np.float32), np.asarray(bb1, np.float32)],
                 axis=1))
    Wb1f = np.ascontiguousarray(
        np.asarray(Wb1, np.float32).reshape(2, 128, DMLP).transpose(1, 0, 2))
    # bb2T[p, l2c, l1] = bb2[l1*L + l2c*128 + p]
    bb2T = np.ascontiguousarray(
        np.asarray(bb2, f16).reshape(L, NC, 128).transpose(2, 1, 0))
    Wb2_16 = np.asarray(Wb2, f16)

    def tr_in(x):
        # [4, l, din] -> [4, p(128), kt, l]
        xt = np.asarray(x, f16).transpose(0, 2, 1)
        return np.ascontiguousarray(
            xt.reshape(BPC, KT, 128, L).transpose(0, 2, 1, 3))

    in_maps = []
    for i in range(N_CORES):
        sl = slice(BPC * i, BPC * (i + 1))
        # mask transposed: mkT[s][p, mc, l1] = mask[s][l1, mc*128+p]
        mkT = np.asarray(atten_mask[sl], np.uint8).transpose(0, 2, 1)
        in_maps.append({
            "qT": tr_in(query[sl]),
            "kT": tr_in(key[sl]),
            "vT": tr_in(value[sl]),
            "mkT": np.ascontiguousarray(
                mkT.reshape(BPC, NC, 128, L).transpose(0, 2, 1, 3)),
            "sfT": sfT,
            "wqkv": wqkv,
            "bias4": bias4,
            "Wb1": Wb1f,
            "Wb2s": np.ascontiguousarray(Wb2_16[:, NSH * i: NSH * (i + 1)]),
            "bb2T": bb2T,
        })
    return in_maps


def kernel(**inputs) -> np.ndarray:
    from concourse import bass_utils
    nc = _build()
    in_maps = _prep_in_maps(**inputs)
    res = bass_utils.run_bass_kernel_spmd(
        nc, in_maps, core_ids=list(range(N_CORES)))
    return np.concatenate([r["out"] for r in res.results],
                          axis=0).astype(np.float32)
